# revision 9
# baseline (speedup 1.0000x reference)
"""Deformable-attention Bass kernel for Trainium2 (8 NeuronCores), v2.

Same math restructuring as v1 (12-pixel window blend, vp folded into op MLP),
plus:
  - 3-column x-window via unaligned i32 indirect gathers on a [H*W, 256]
    channels-last bev (25% fewer gather bytes / blend elements than the
    4-col aligned variant); window base = floor(center+0.5)-1 on both axes,
    so dx, dy in [0.5, 1.5) and every hat weight is relu-affine (no abs):
      h0 = relu(1-d), h2 = relu(d-1), h1 = 1 - |d-1| = 1 - (h0+h2)
  - per-pixel channel order transposed to (c, h) on host so the per-(i,j,h)
    blend weights broadcast over c with a step-1 innermost h-run: every big
    DVE tensor_tensor runs in 2x bf16 mode
  - so2/aw2 computed token-major directly on PE (lhsT = hidden activations,
    rhs = weights), killing the so/aw transpose+copy pass
  - attn token-major -> channel-major via DMA xbar transposes issued on SP
  - gathers batched per quarter (SWDGE fixed cost amortized over 1536
    descriptors per instruction)
  - softmax denominator folded into v12 as a bf16 reciprocal multiply
"""

import numpy as np

B, N, D, NH, NP, H, W = 4, 4096, 256, 8, 8, 256, 256
HD = D // NH
NCORES = 8
T = B * N // NCORES      # 2048 tokens per core
NT = T // 128            # 16 token tiles
NQ = 4                   # quarters
TPQ = NT // NQ           # tiles per quarter

MLP_BF16 = True
PATCH_BF16 = True

# tiles whose blend products / adds-tree run on gpsimd instead of DVE
POOL_PROD_TILES = frozenset({5, 11})
POOL_TREE_TILES = frozenset({8})

_CACHE = {}


# ----------------------------------------------------------------- host prep
def _bf16():
    import ml_dtypes
    return ml_dtypes.bfloat16


def _mm_np_dtype():
    return _bf16() if MLP_BF16 else np.float32


def _pack_w(w):
    """[256, O] weight -> [128, 2*O] sbuf layout: [p, k*O+o] = w[k*128+p, o]."""
    K, O = w.shape
    assert K == 256
    return np.ascontiguousarray(
        w.reshape(2, 128, O).transpose(1, 0, 2).reshape(128, 2 * O)
    ).astype(_mm_np_dtype())


def _pack_b(b):
    """[O] bias -> [128, ceil(O/128)] per-partition columns (fp32)."""
    O = b.shape[0]
    if O % 128:
        b = np.pad(b, (0, 128 - O % 128))
    c = b.shape[0] // 128
    return np.ascontiguousarray(b.reshape(c, 128).T).astype(np.float32)


def _host_prep(inputs):
    q = np.asarray(inputs["ba_query"], np.float32)        # [B, N, D]
    ref = np.asarray(inputs["ref_pos"], np.float64)       # [B, N, 2]
    bev = np.asarray(inputs["bev_feat"], np.float32)      # [B, D, H, W]

    f64 = np.float64
    so_w1 = np.asarray(inputs["so_w1"], f64)
    so_b1 = np.asarray(inputs["so_b1"], f64)
    so_w2 = np.asarray(inputs["so_w2"], f64)
    so_b2 = np.asarray(inputs["so_b2"], f64)
    aw_w1 = np.asarray(inputs["aw_w1"], f64)
    aw_b1 = np.asarray(inputs["aw_b1"], f64)
    aw_w2 = np.asarray(inputs["aw_w2"], f64)
    aw_b2 = np.asarray(inputs["aw_b2"], f64)
    vp_w = np.asarray(inputs["vp_w"], f64)
    vp_b = np.asarray(inputs["vp_b"], f64)
    op_w1 = np.asarray(inputs["op_w1"], f64)
    op_b1 = np.asarray(inputs["op_b1"], f64)
    op_w2 = np.asarray(inputs["op_w2"], f64)
    op_b2 = np.asarray(inputs["op_b2"], f64)

    # sampling-offset head: de-interleave (x, y) cols, scale to pixels
    # x_pix = xc + 0.5*so_x ; y_pix = yc - 0.5*so_y (y-flip folded)
    w_so2 = np.concatenate([so_w2[:, 0::2] * 0.5, so_w2[:, 1::2] * -0.5], axis=1)
    b_so2 = np.concatenate([so_b2[0::2] * 0.5, so_b2[1::2] * -0.5], axis=0)

    # fold value projection into op MLP; permute rows to the on-device
    # (c, h) channel order of attn
    BD = np.zeros((D, D), f64)
    for h in range(NH):
        BD[h * HD:(h + 1) * HD, h * HD:(h + 1) * HD] = vp_w.T
    w_op1 = BD @ op_w1
    b_op1 = op_b1 + np.tile(vp_b, NH) @ op_w1
    src = (np.arange(D) % NH) * HD + np.arange(D) // NH   # ch'=c*8+h -> h*32+c
    w_op1 = w_op1[src, :]

    mmdt = _mm_np_dtype()
    weight_map = {
        "w_so1": _pack_w(so_w1), "b_so1": _pack_b(so_b1),
        "w_aw1": _pack_w(aw_w1), "b_aw1": _pack_b(aw_b1),
        "w_so2": _pack_w(w_so2),
        "w_aw2": _pack_w(aw_w2),
        "brow_so2": np.ascontiguousarray(b_so2.reshape(1, 128)).astype(mmdt),
        "brow_aw2": np.ascontiguousarray(aw_b2.reshape(1, 64)).astype(mmdt),
        "w_op1": _pack_w(w_op1), "b_op1": _pack_b(b_op1),
        "w_op2": _pack_w(op_w2), "b_op2": _pack_b(op_b2),
    }

    # channels-last BEV with per-pixel (c, h) order -> [H*W, 256] rows
    pdt = _bf16() if PATCH_BF16 else np.float32
    bev_g = np.ascontiguousarray(
        bev.reshape(B, NH, HD, H, W).transpose(0, 3, 4, 2, 1)
        .reshape(B, H * W, D)).astype(pdt)

    # per-token window geometry (depends only on ref_pos)
    xc = (ref[..., 0] + 1.0) * (W / 2) - 0.5                   # [B, N]
    yc = (1.0 - ref[..., 1]) * (H / 2) - 0.5
    bx = np.clip(np.floor(xc + 0.5).astype(np.int64) - 1, 0, W - 3)
    by = np.clip(np.floor(yc + 0.5).astype(np.int64) - 1, 0, H - 3)
    cx = (xc - bx).astype(np.float32)                          # in [0.5, 1.5)
    cy = (yc - by).astype(np.float32)
    ks = np.arange(3).reshape(1, 1, 3)
    rows = ((by[..., None] + ks) * W + bx[..., None]).astype(np.int32)

    in_maps = []
    for c in range(NCORES):
        b, half = divmod(c, 2)
        sl = slice(half * T, (half + 1) * T)
        qs = q[b, sl].T                                         # [256, T]
        q_dev = np.ascontiguousarray(
            qs.reshape(2, 128, T).transpose(1, 0, 2)).astype(mmdt)

        idx_all = np.ascontiguousarray(
            rows[b, sl].reshape(NT, 128, 3).transpose(1, 0, 2)
            .reshape(128, NT * 3)).astype(np.int32)

        # cx/cy expanded over the 64 (h,p) slots, bf16, token-major tiles
        cxe = np.repeat(cx[b, sl].reshape(NT, 128, 1), 64, axis=2)
        cye = np.repeat(cy[b, sl].reshape(NT, 128, 1), 64, axis=2)
        cxe = np.ascontiguousarray(
            cxe.transpose(1, 0, 2).reshape(128, NT * 64)).astype(_bf16())
        cye = np.ascontiguousarray(
            cye.transpose(1, 0, 2).reshape(128, NT * 64)).astype(_bf16())

        m = {"q": q_dev, "bev": bev_g[b], "idx": idx_all, "cxe": cxe, "cye": cye}
        m.update(weight_map)
        in_maps.append(m)
    return in_maps


# ------------------------------------------------------------- device kernel
def _build_nc(repeat=1):
    import concourse.bass as bass
    import concourse.tile as tile
    from concourse import bacc, mybir
    from concourse.bass import ts
    from concourse.masks import make_identity
    from contextlib import ExitStack

    f32 = mybir.dt.float32
    bf16 = mybir.dt.bfloat16
    i32 = mybir.dt.int32
    mmdt = bf16 if MLP_BF16 else f32
    pdt = bf16 if PATCH_BF16 else f32
    AF = mybir.ActivationFunctionType
    OP = mybir.AluOpType

    nc = bacc.Bacc("TRN2", target_bir_lowering=False, debug=False)

    d_q = nc.dram_tensor("q", [128, 2, T], mmdt, kind="ExternalInput")
    d_bev = nc.dram_tensor("bev", [H * W, D], pdt, kind="ExternalInput")
    d_idx = nc.dram_tensor("idx", [128, NT * 3], i32, kind="ExternalInput")
    d_cxe = nc.dram_tensor("cxe", [128, NT * 64], bf16, kind="ExternalInput")
    d_cye = nc.dram_tensor("cye", [128, NT * 64], bf16, kind="ExternalInput")
    dw = {}
    for nm, sh, dt_ in [
        ("w_so1", [128, 512], mmdt), ("b_so1", [128, 2], f32),
        ("w_aw1", [128, 512], mmdt), ("b_aw1", [128, 2], f32),
        ("w_so2", [128, 256], mmdt), ("brow_so2", [1, 128], mmdt),
        ("w_aw2", [128, 128], mmdt), ("brow_aw2", [1, 64], mmdt),
        ("w_op1", [128, 512], mmdt), ("b_op1", [128, 2], f32),
        ("w_op2", [128, 512], mmdt), ("b_op2", [128, 2], f32),
    ]:
        dw[nm] = nc.dram_tensor(nm, sh, dt_, kind="ExternalInput")
    d_out = nc.dram_tensor("out", [2, 128, T], f32, kind="ExternalOutput")

    def mk_ap(base_ap, extra_off, frees):
        return bass.AP(tensor=base_ap.tensor, offset=base_ap.offset + extra_off,
                       ap=[base_ap.ap[0]] + [list(f) for f in frees])

    with tile.TileContext(nc) as tc, ExitStack() as ctx:
        const = ctx.enter_context(tc.tile_pool(name="const", bufs=1))
        pers = ctx.enter_context(tc.tile_pool(name="pers", bufs=1))
        psmm = ctx.enter_context(tc.tile_pool(name="psmm", bufs=3, space="PSUM"))
        ptrp = ctx.enter_context(tc.tile_pool(name="ptrp", bufs=1, space="PSUM"))
        pstr = ctx.enter_context(tc.tile_pool(name="pstr", bufs=2, space="PSUM"))

        # ---- constants (idx first: gathers depend on it; then chunk-0 MLP
        # inputs so the pipeline ramps; op weights last)
        idx_sb = const.tile([128, NT * 3], i32)
        nc.sync.dma_start(idx_sb[:], d_idx[:])
        w_sb = {}
        for nm in dw:
            tl = const.tile(list(dw[nm].shape), dw[nm].dtype, tag=nm)
            w_sb[nm] = tl
        q_sb = pers.tile([128, 2, T], mmdt)
        nc.scalar.dma_start(q_sb[:, :, 0:512], d_q[:, :, 0:512])
        early = ["w_so1", "b_so1", "w_aw1", "b_aw1", "w_so2", "brow_so2",
                 "w_aw2", "brow_aw2"]
        for nm in early:
            nc.sync.dma_start(w_sb[nm][:], dw[nm][:])
        cxe_sb = const.tile([128, NT * 64], bf16)
        nc.sync.dma_start(cxe_sb[:], d_cxe[:])
        cye_sb = const.tile([128, NT * 64], bf16)
        nc.sync.dma_start(cye_sb[:], d_cye[:])
        for qd in range(1, NQ):
            nc.sync.dma_start(q_sb[:, :, ts(qd, 512)], d_q[:, :, ts(qd, 512)])
        for nm in dw:
            if nm not in early:
                nc.sync.dma_start(w_sb[nm][:], dw[nm][:])
        ones1 = const.tile([1, 128], mmdt, tag="ones1")
        nc.vector.memset(ones1[:], 1.0)
        negone = const.tile([128, 1], f32, tag="negone")
        nc.vector.memset(negone[:], -1.0)
        ident = const.tile([128, 128], mmdt, tag="ident")
        make_identity(nc, ident[:])
        ones512 = const.tile([1, 512], mmdt, tag="ones512")
        nc.vector.memset(ones512[:], 1.0)

        # ---- persistent activations
        h1so = pers.tile([128, 2, T], mmdt)
        h1aw = pers.tile([128, 2, T], mmdt)
        h1op = pers.tile([128, 2, T], mmdt)
        soT = pers.tile([128, NT * 128], bf16)    # [t, (x64|y64)] pixel offsets
        ew = pers.tile([128, NT * 64], bf16)      # exp(aw logits), (h,p)
        sume = pers.tile([128, NT * 8], f32)
        rec = pers.tile([128, NT * 8], bf16)
        v12 = pers.tile([128, NT * 72], bf16)     # (tile, i, j, h) raw
        v12n = pers.tile([128, NT * 72], bf16)    # normalized
        attn_cm = pers.tile([128, 2, T], mmdt)    # channel-major (c,h)-perm
        out_sb = pers.tile([128, 2, T], f32)

        if repeat > 1:
            ctx.enter_context(tc.For_i(0, repeat, 1))

        # ---- batched gathers (SWDGE on gpsimd); emitted staggered so their
        # transfer timelines don't clog the Pool queue ahead of v12 trees
        patches = ctx.enter_context(tc.tile_pool(name="patch", bufs=1))
        patch_q = []

        def gather(g):
            # 2 token tiles; one single-offset indirect DMA per (tile, window
            # row) -- the only SWDGE addressing pattern verified on hardware
            patch = patches.tile([128, 2 * 3 * 768], pdt, tag=f"patch{g}")
            for c in range(6):
                col = g * 6 + c
                nc.gpsimd.indirect_dma_start(
                    out=patch[:, c * 768:(c + 1) * 768], out_offset=None,
                    in_=d_bev[:],
                    in_offset=bass.IndirectOffsetOnAxis(
                        ap=idx_sb[:, col:col + 1], axis=0))
            patch_q.append(patch)

        gather(0)

        # ---- pipelined stages
        phg = ctx.enter_context(tc.tile_pool(name="phg", bufs=2))
        prodp = ctx.enter_context(tc.tile_pool(name="prod", bufs=3))
        treep = ctx.enter_context(tc.tile_pool(name="tree", bufs=2))
        attnp = ctx.enter_context(tc.tile_pool(name="attn", bufs=3))

        def mlp1(qd, wname, bname, h1):
            """First-layer MLP chunk (512 tokens), channel-major."""
            wt, bt = w_sb[wname], w_sb[bname]
            for m in range(2):
                ps = psmm.tile([128, 512], f32, tag="mmps")
                for kk in range(2):
                    nc.tensor.matmul(
                        ps[:],
                        lhsT=wt[:, kk * 256 + m * 128: kk * 256 + m * 128 + 128],
                        rhs=q_sb[:, kk, ts(qd, 512)],
                        start=(kk == 0), stop=(kk == 1))
                nc.scalar.activation(
                    out=h1[:, m, ts(qd, 512)], in_=ps[:],
                    func=AF.Relu, bias=bt[:, m:m + 1], scale=1.0)

        def so2aw2(j):
            """Token-major so2/aw2 for tile j (lhsT = hidden activations)."""
            ps = pstr.tile([128, 128], f32, tag="so2ps")
            pa = pstr.tile([128, 64], f32, tag="aw2ps")
            nc.tensor.matmul(ps[:], lhsT=ones1[:1, :],
                             rhs=w_sb["brow_so2"][:1, :], start=True, stop=False)
            nc.tensor.matmul(pa[:], lhsT=ones1[:1, :],
                             rhs=w_sb["brow_aw2"][:1, :], start=True, stop=False)
            for kk in range(2):
                last = kk == 1
                nc.tensor.matmul(ps[:], lhsT=h1so[:, kk, ts(j, 128)],
                                 rhs=w_sb["w_so2"][:, kk * 128:(kk + 1) * 128],
                                 start=False, stop=last)
                nc.tensor.matmul(pa[:], lhsT=h1aw[:, kk, ts(j, 128)],
                                 rhs=w_sb["w_aw2"][:, kk * 64:(kk + 1) * 64],
                                 start=False, stop=last)
            nc.scalar.activation(out=soT[:, ts(j, 128)], in_=ps[:],
                                 func=AF.Identity, bias=0.0, scale=1.0)
            nc.scalar.activation(out=ew[:, ts(j, 64)], in_=pa[:],
                                 func=AF.Exp, bias=0.0, scale=1.0)

        def geometry(qd):
            """Per-quarter v12 construction (4 tiles, 256 (tile,h,p) cols)."""
            C = TPQ * 64
            o = qd * C
            dx = phg.tile([128, C], bf16, tag="dx")
            dy = phg.tile([128, C], bf16, tag="dy")
            soTa = soT[:]
            nc.vector.tensor_tensor(
                out=dx[:].rearrange("p (a b) -> p a b", b=64),
                in0=mk_ap(soTa, qd * TPQ * 128, [[128, TPQ], [1, 64]]),
                in1=mk_ap(cxe_sb[:], o, [[64, TPQ], [1, 64]]), op=OP.add)
            nc.vector.tensor_tensor(
                out=dy[:].rearrange("p (a b) -> p a b", b=64),
                in0=mk_ap(soTa, qd * TPQ * 128 + 64, [[128, TPQ], [1, 64]]),
                in1=mk_ap(cye_sb[:], o, [[64, TPQ], [1, 64]]), op=OP.add)

            # x-hats straight into nwx slots [t, (tile, j, h, p)]
            nwx = phg.tile([128, TPQ * 192], bf16, tag="nwx")
            nwxa = nwx[:]
            dx2 = dx[:].rearrange("p (a b) -> p a b", b=64)
            nc.scalar.activation(
                out=mk_ap(nwxa, 0, [[192, TPQ], [1, 64]]), in_=dx2,
                func=AF.Relu, bias=1.0, scale=-1.0)          # h0 = relu(1-dx)
            nc.scalar.activation(
                out=mk_ap(nwxa, 128, [[192, TPQ], [1, 64]]), in_=dx2,
                func=AF.Relu, bias=negone[:], scale=1.0)     # h2 = relu(dx-1)
            u = phg.tile([128, C], bf16, tag="absu")
            nc.scalar.activation(out=u[:], in_=dx[:], func=AF.Abs,
                                 bias=negone[:], scale=1.0)
            nc.scalar.activation(
                out=mk_ap(nwxa, 64, [[192, TPQ], [1, 64]]),
                in_=u[:].rearrange("p (a b) -> p a b", b=64),
                func=AF.Identity, bias=1.0, scale=-1.0)      # h1 = 1-|dx-1|

            # y-hats folded with exp(aw)
            hy0 = phg.tile([128, C], bf16, tag="hy0")
            hy2 = phg.tile([128, C], bf16, tag="hy2")
            nc.scalar.activation(out=hy0[:], in_=dy[:], func=AF.Relu,
                                 bias=1.0, scale=-1.0)
            nc.scalar.activation(out=hy2[:], in_=dy[:], func=AF.Relu,
                                 bias=negone[:], scale=1.0)
            ewq = ew[:, o:o + C]
            ewy0 = phg.tile([128, C], bf16, tag="ewy0")
            ewy1 = phg.tile([128, C], bf16, tag="ewy1")
            ewy2 = phg.tile([128, C], bf16, tag="ewy2")
            nc.vector.tensor_tensor(out=ewy0[:], in0=hy0[:], in1=ewq, op=OP.mult)
            nc.vector.tensor_tensor(out=ewy2[:], in0=hy2[:], in1=ewq, op=OP.mult)
            tmp = phg.tile([128, C], bf16, tag="ewytmp")
            nc.vector.tensor_tensor(out=tmp[:], in0=ewy0[:], in1=ewy2[:],
                                    op=OP.add)
            nc.vector.tensor_tensor(out=ewy1[:], in0=ewq, in1=tmp[:],
                                    op=OP.subtract)

            # softmax denominator and reciprocal
            nc.vector.tensor_reduce(
                out=sume[:, qd * 32:(qd + 1) * 32],
                in_=ewq.rearrange("p (g q) -> p g q", q=NP),
                axis=mybir.AxisListType.X, op=OP.add)
            with nc.allow_low_precision(reason="softmax recip folded to bf16"):
                nc.vector.reciprocal(rec[:, qd * 32:(qd + 1) * 32],
                                     sume[:, qd * 32:(qd + 1) * 32])

            # pr_i = ewy_i (bcast over j) * nwx  -> v12 via gpsimd adds-tree
            v12a = v12[:]
            for i, ewy in enumerate((ewy0, ewy1, ewy2)):
                pr = phg.tile([128, TPQ * 192], bf16, tag=f"pr{i}")
                nc.vector.tensor_tensor(
                    out=pr[:].rearrange("p (a j b) -> p a j b", a=TPQ, j=3),
                    in0=mk_ap(ewy[:], 0, [[64, TPQ], [0, 3], [1, 64]]),
                    in1=mk_ap(nwxa, 0, [[192, TPQ], [64, 3], [1, 64]]),
                    op=OP.mult)
                pra = pr[:]
                t1 = phg.tile([128, TPQ * 96], f32, tag=f"prt1_{i}")
                nc.gpsimd.tensor_tensor(
                    out=t1[:].rearrange("p (g q) -> p g q", q=4),
                    in0=mk_ap(pra, 0, [[8, TPQ * 24], [1, 4]]),
                    in1=mk_ap(pra, 4, [[8, TPQ * 24], [1, 4]]), op=OP.add)
                t1a = t1[:]
                t2 = phg.tile([128, TPQ * 48], f32, tag=f"prt2_{i}")
                nc.gpsimd.tensor_tensor(
                    out=t2[:].rearrange("p (g q) -> p g q", q=2),
                    in0=mk_ap(t1a, 0, [[4, TPQ * 24], [1, 2]]),
                    in1=mk_ap(t1a, 2, [[4, TPQ * 24], [1, 2]]), op=OP.add)
                t2a = t2[:]
                nc.gpsimd.tensor_tensor(
                    out=mk_ap(v12a, qd * TPQ * 72 + i * 24,
                              [[72, TPQ], [1, 24]]),
                    in0=mk_ap(t2a, 0, [[2, TPQ * 24], [1, 1]]),
                    in1=mk_ap(t2a, 1, [[2, TPQ * 24], [1, 1]]), op=OP.add)

            # normalize: v12n = v12 * rec  (bf16 2x)
            nc.vector.tensor_tensor(
                out=v12n[:, qd * TPQ * 72:(qd + 1) * TPQ * 72].rearrange(
                    "p (a s b) -> p a s b", a=TPQ, s=9),
                in0=v12[:, qd * TPQ * 72:(qd + 1) * TPQ * 72].rearrange(
                    "p (a s b) -> p a s b", a=TPQ, s=9),
                in1=mk_ap(rec[:], qd * 32, [[8, TPQ], [0, 9], [1, 8]]),
                op=OP.mult)

        def blend(j):
            """12-pixel weighted blend for tile j -> attn_cm via DMA transpose."""
            jq = j % 2
            patch = patch_q[j // 2]
            peng = nc.gpsimd if j in POOL_PROD_TILES else nc.vector
            teng = nc.gpsimd if j in POOL_TREE_TILES else nc.vector
            prodb = prodp.tile([128, 2304], pdt, tag="prodb")
            peng.tensor_tensor(
                out=prodb[:].rearrange("p (s c b) -> p s c b", s=9, c=32),
                in0=patch[:, jq * 2304:(jq + 1) * 2304].rearrange(
                    "p (s c b) -> p s c b", s=9, c=32),
                in1=mk_ap(v12n[:], j * 72, [[8, 9], [0, 32], [1, 8]]),
                op=OP.mult)
            t1 = treep.tile([128, 1024], pdt, tag="t1")
            teng.tensor_tensor(out=t1[:], in0=prodb[:, 0:1024],
                               in1=prodb[:, 1024:2048], op=OP.add)
            t2 = treep.tile([128, 512], pdt, tag="t2")
            teng.tensor_tensor(out=t2[:], in0=t1[:, 0:512],
                               in1=t1[:, 512:1024], op=OP.add)
            t3 = treep.tile([128, 256], pdt, tag="t3")
            teng.tensor_tensor(out=t3[:], in0=t2[:, 0:256],
                               in1=t2[:, 256:512], op=OP.add)
            attn = attnp.tile([128, 256], pdt, tag="attn")
            teng.tensor_tensor(out=attn[:], in0=t3[:],
                               in1=prodb[:, 2048:2304], op=OP.add)
            if j >= 3 * TPQ:
                # tail tiles: PE transpose + ACT copy (short latency, and
                # keeps the PE p-state warm going into the final op MLP)
                for m in range(2):
                    pt = ptrp.tile([128, 128], pdt, tag="trps")
                    nc.tensor.transpose(pt[:], attn[:, ts(m, 128)], ident[:])
                    nc.vector.tensor_copy(attn_cm[:, m, ts(j, 128)], pt[:])
            else:
                for m in range(2):
                    nc.sync.dma_start_transpose(
                        out=attn_cm[:, m, ts(j, 128)], in_=attn[:, ts(m, 128)])

        def opmlp(t0, nt, dve_acts=False):
            """Output MLP for tokens [t0*128, (t0+nt)*128), channel-major.

            dve_acts: run the bias/activation stage on DVE tensor_scalar
            instead of ACT (for the tail, where DVE is idle and ACT is the
            critical chain)."""
            lo, sz = t0 * 128, nt * 128
            sl = slice(lo, lo + sz)
            for m in range(2):
                ps = psmm.tile([128, 512], f32, tag="mmps")
                for kk in range(2):
                    nc.tensor.matmul(
                        ps[:, 0:sz],
                        lhsT=w_sb["w_op1"][:, kk * 256 + m * 128:
                                           kk * 256 + m * 128 + 128],
                        rhs=attn_cm[:, kk, sl],
                        start=(kk == 0), stop=(kk == 1))
                nc.scalar.activation(
                    out=h1op[:, m, sl], in_=ps[:, 0:sz],
                    func=AF.Relu, bias=w_sb["b_op1"][:, m:m + 1], scale=1.0)
            for m in range(2):
                ps = psmm.tile([128, 512], f32, tag="mmps")
                for kk in range(2):
                    nc.tensor.matmul(
                        ps[:, 0:sz],
                        lhsT=w_sb["w_op2"][:, kk * 256 + m * 128:
                                           kk * 256 + m * 128 + 128],
                        rhs=h1op[:, kk, sl],
                        start=(kk == 0), stop=(kk == 1))
                nc.scalar.activation(
                    out=out_sb[:, m, sl], in_=ps[:, 0:sz],
                    func=AF.Identity, bias=w_sb["b_op2"][:, m:m + 1],
                    scale=1.0)
            nc.sync.dma_start(
                d_out[:, :, sl].rearrange("k p t -> p k t"),
                out_sb[:, :, sl])

        def stage_a(qd):
            mlp1(qd, "w_so1", "b_so1", h1so)
            mlp1(qd, "w_aw1", "b_aw1", h1aw)
            for j in range(qd * TPQ, (qd + 1) * TPQ):
                so2aw2(j)

        # PE p-state warmup: two throwaway matmuls so the first MLP chunk
        # doesn't run at the cold clock
        wps = psmm.tile([128, 512], f32, tag="mmps")
        nc.tensor.matmul(wps[:], lhsT=ones1[:1, :], rhs=ones512[:1, :],
                         start=True, stop=False)
        nc.tensor.matmul(wps[:], lhsT=ones1[:1, :], rhs=ones512[:1, :],
                         start=False, stop=True)

        # software-pipelined emission: keep every engine queue one stage
        # ahead so in-order queues never head-of-line block
        stage_a(0)
        geometry(0)
        stage_a(1)
        gather(1)
        gather(2)
        for j in range(0, TPQ):
            blend(j)
        geometry(1)
        stage_a(2)
        gather(3)
        gather(4)
        for j in range(TPQ, 2 * TPQ):
            blend(j)
        geometry(2)
        opmlp(0, 4)
        stage_a(3)
        gather(5)
        gather(6)
        for j in range(2 * TPQ, 3 * TPQ):
            blend(j)
        geometry(3)
        opmlp(4, 4)
        gather(7)
        for j in range(3 * TPQ, 4 * TPQ):
            blend(j)
            if j == 3 * TPQ + 1:
                opmlp(8, 4)
        opmlp(12, 2)
        opmlp(14, 2, dve_acts=True)

    nc.compile()
    return nc


def get_nc():
    if "nc" not in _CACHE:
        _CACHE["nc"] = _build_nc()
    return _CACHE["nc"]


# ------------------------------------------------------------------- launch
def kernel(**inputs):
    from concourse import bass_utils

    nc = get_nc()
    in_maps = _host_prep(inputs)
    res = bass_utils.run_bass_kernel_spmd(
        nc, in_maps, core_ids=list(range(NCORES)))
    out = np.empty((B, N, D), np.float32)
    for c in range(NCORES):
        b, half = divmod(c, 2)
        o = np.asarray(res.results[c]["out"]).reshape(D, T)
        out[b, half * T:(half + 1) * T, :] = o.T
    return out


# revision 10
# speedup vs baseline: 1.8074x; 1.8074x over previous
"""Deformable-attention Bass kernel for Trainium2 (8 NeuronCores), v2.

Same math restructuring as v1 (12-pixel window blend, vp folded into op MLP),
plus:
  - 3-column x-window via unaligned i32 indirect gathers on a [H*W, 256]
    channels-last bev (25% fewer gather bytes / blend elements than the
    4-col aligned variant); window base = floor(center+0.5)-1 on both axes,
    so dx, dy in [0.5, 1.5) and every hat weight is relu-affine (no abs):
      h0 = relu(1-d), h2 = relu(d-1), h1 = 1 - |d-1| = 1 - (h0+h2)
  - per-pixel channel order transposed to (c, h) on host so the per-(i,j,h)
    blend weights broadcast over c with a step-1 innermost h-run: every big
    DVE tensor_tensor runs in 2x bf16 mode
  - so2/aw2 computed token-major directly on PE (lhsT = hidden activations,
    rhs = weights), killing the so/aw transpose+copy pass
  - attn token-major -> channel-major via DMA xbar transposes issued on SP
  - gathers batched per quarter (SWDGE fixed cost amortized over 1536
    descriptors per instruction)
  - softmax denominator folded into v12 as a bf16 reciprocal multiply
"""

import numpy as np

B, N, D, NH, NP, H, W = 4, 4096, 256, 8, 8, 256, 256
HD = D // NH
NCORES = 8
T = B * N // NCORES      # 2048 tokens per core
NT = T // 128            # 16 token tiles
NQ = 4                   # quarters
TPQ = NT // NQ           # tiles per quarter

MLP_BF16 = True
PATCH_BF16 = True

# tiles whose blend products / adds-tree run on gpsimd instead of DVE
POOL_PROD_TILES = frozenset({5, 11})
POOL_TREE_TILES = frozenset({8})

_CACHE = {}


# ----------------------------------------------------------------- host prep
def _bf16():
    import ml_dtypes
    return ml_dtypes.bfloat16


def _mm_np_dtype():
    return _bf16() if MLP_BF16 else np.float32


def _pack_w(w):
    """[256, O] weight -> [128, 2*O] sbuf layout: [p, k*O+o] = w[k*128+p, o]."""
    K, O = w.shape
    assert K == 256
    return np.ascontiguousarray(
        w.reshape(2, 128, O).transpose(1, 0, 2).reshape(128, 2 * O)
    ).astype(_mm_np_dtype())


def _pack_b(b):
    """[O] bias -> [128, ceil(O/128)] per-partition columns (fp32)."""
    O = b.shape[0]
    if O % 128:
        b = np.pad(b, (0, 128 - O % 128))
    c = b.shape[0] // 128
    return np.ascontiguousarray(b.reshape(c, 128).T).astype(np.float32)


def _host_prep(inputs):
    q = np.asarray(inputs["ba_query"], np.float32)        # [B, N, D]
    ref = np.asarray(inputs["ref_pos"], np.float64)       # [B, N, 2]
    bev = np.asarray(inputs["bev_feat"], np.float32)      # [B, D, H, W]

    f64 = np.float64
    so_w1 = np.asarray(inputs["so_w1"], f64)
    so_b1 = np.asarray(inputs["so_b1"], f64)
    so_w2 = np.asarray(inputs["so_w2"], f64)
    so_b2 = np.asarray(inputs["so_b2"], f64)
    aw_w1 = np.asarray(inputs["aw_w1"], f64)
    aw_b1 = np.asarray(inputs["aw_b1"], f64)
    aw_w2 = np.asarray(inputs["aw_w2"], f64)
    aw_b2 = np.asarray(inputs["aw_b2"], f64)
    vp_w = np.asarray(inputs["vp_w"], f64)
    vp_b = np.asarray(inputs["vp_b"], f64)
    op_w1 = np.asarray(inputs["op_w1"], f64)
    op_b1 = np.asarray(inputs["op_b1"], f64)
    op_w2 = np.asarray(inputs["op_w2"], f64)
    op_b2 = np.asarray(inputs["op_b2"], f64)

    # sampling-offset head: de-interleave (x, y) cols, scale to pixels
    # x_pix = xc + 0.5*so_x ; y_pix = yc - 0.5*so_y (y-flip folded)
    w_so2 = np.concatenate([so_w2[:, 0::2] * 0.5, so_w2[:, 1::2] * -0.5], axis=1)
    b_so2 = np.concatenate([so_b2[0::2] * 0.5, so_b2[1::2] * -0.5], axis=0)

    # fold value projection into op MLP; permute rows to the on-device
    # (c, h) channel order of attn
    BD = np.zeros((D, D), f64)
    for h in range(NH):
        BD[h * HD:(h + 1) * HD, h * HD:(h + 1) * HD] = vp_w.T
    w_op1 = BD @ op_w1
    b_op1 = op_b1 + np.tile(vp_b, NH) @ op_w1
    src = (np.arange(D) % NH) * HD + np.arange(D) // NH   # ch'=c*8+h -> h*32+c
    w_op1 = w_op1[src, :]

    mmdt = _mm_np_dtype()
    weight_map = {
        "w_so1": _pack_w(so_w1), "b_so1": _pack_b(so_b1),
        "w_aw1": _pack_w(aw_w1), "b_aw1": _pack_b(aw_b1),
        "w_so2": _pack_w(w_so2),
        "w_aw2": _pack_w(aw_w2),
        "brow_so2": np.ascontiguousarray(b_so2.reshape(1, 128)).astype(mmdt),
        "brow_aw2": np.ascontiguousarray(aw_b2.reshape(1, 64)).astype(mmdt),
        "w_op1": _pack_w(w_op1), "b_op1": _pack_b(b_op1),
        "w_op2": _pack_w(op_w2), "b_op2": _pack_b(op_b2),
    }

    # channels-last BEV with per-pixel (c, h) order -> [H*W, 256] rows
    pdt = _bf16() if PATCH_BF16 else np.float32
    bev_g = np.ascontiguousarray(
        bev.reshape(B, NH, HD, H, W).transpose(0, 3, 4, 2, 1)
        .reshape(B, H * W, D)).astype(pdt)

    # per-token window geometry (depends only on ref_pos)
    xc = (ref[..., 0] + 1.0) * (W / 2) - 0.5                   # [B, N]
    yc = (1.0 - ref[..., 1]) * (H / 2) - 0.5
    bx = np.clip(np.floor(xc + 0.5).astype(np.int64) - 1, 0, W - 3)
    by = np.clip(np.floor(yc + 0.5).astype(np.int64) - 1, 0, H - 3)
    cx = (xc - bx).astype(np.float32)                          # in [0.5, 1.5)
    cy = (yc - by).astype(np.float32)
    ks = np.arange(3).reshape(1, 1, 3)
    rows = ((by[..., None] + ks) * W + bx[..., None]).astype(np.int32)

    in_maps = []
    for c in range(NCORES):
        b, half = divmod(c, 2)
        sl = slice(half * T, (half + 1) * T)
        qs = q[b, sl].T                                         # [256, T]
        q_dev = np.ascontiguousarray(
            qs.reshape(2, 128, T).transpose(1, 0, 2)).astype(mmdt)

        idx_all = np.ascontiguousarray(
            rows[b, sl].reshape(NT, 128, 3).transpose(1, 0, 2)
            .reshape(128, NT * 3)).astype(np.int32)

        # cx/cy expanded over the 64 (h,p) slots, bf16, token-major tiles
        cxe = np.repeat(cx[b, sl].reshape(NT, 128, 1), 64, axis=2)
        cye = np.repeat(cy[b, sl].reshape(NT, 128, 1), 64, axis=2)
        cxe = np.ascontiguousarray(
            cxe.transpose(1, 0, 2).reshape(128, NT * 64)).astype(_bf16())
        cye = np.ascontiguousarray(
            cye.transpose(1, 0, 2).reshape(128, NT * 64)).astype(_bf16())

        m = {"q": q_dev, "bev": bev_g[b], "idx": idx_all, "cxe": cxe, "cye": cye}
        m.update(weight_map)
        in_maps.append(m)
    return in_maps


# ------------------------------------------------------------- device kernel
def _build_nc(repeat=1):
    import concourse.bass as bass
    import concourse.tile as tile
    from concourse import bacc, mybir
    from concourse.bass import ts
    from concourse.masks import make_identity
    from contextlib import ExitStack

    f32 = mybir.dt.float32
    bf16 = mybir.dt.bfloat16
    i32 = mybir.dt.int32
    mmdt = bf16 if MLP_BF16 else f32
    pdt = bf16 if PATCH_BF16 else f32
    AF = mybir.ActivationFunctionType
    OP = mybir.AluOpType

    nc = bacc.Bacc("TRN2", target_bir_lowering=False, debug=False)

    d_q = nc.dram_tensor("q", [128, 2, T], mmdt, kind="ExternalInput")
    d_bev = nc.dram_tensor("bev", [H * W, D], pdt, kind="ExternalInput")
    d_idx = nc.dram_tensor("idx", [128, NT * 3], i32, kind="ExternalInput")
    d_cxe = nc.dram_tensor("cxe", [128, NT * 64], bf16, kind="ExternalInput")
    d_cye = nc.dram_tensor("cye", [128, NT * 64], bf16, kind="ExternalInput")
    dw = {}
    for nm, sh, dt_ in [
        ("w_so1", [128, 512], mmdt), ("b_so1", [128, 2], f32),
        ("w_aw1", [128, 512], mmdt), ("b_aw1", [128, 2], f32),
        ("w_so2", [128, 256], mmdt), ("brow_so2", [1, 128], mmdt),
        ("w_aw2", [128, 128], mmdt), ("brow_aw2", [1, 64], mmdt),
        ("w_op1", [128, 512], mmdt), ("b_op1", [128, 2], f32),
        ("w_op2", [128, 512], mmdt), ("b_op2", [128, 2], f32),
    ]:
        dw[nm] = nc.dram_tensor(nm, sh, dt_, kind="ExternalInput")
    d_out = nc.dram_tensor("out", [2, 128, T], f32, kind="ExternalOutput")

    def mk_ap(base_ap, extra_off, frees):
        return bass.AP(tensor=base_ap.tensor, offset=base_ap.offset + extra_off,
                       ap=[base_ap.ap[0]] + [list(f) for f in frees])

    with tile.TileContext(nc) as tc, ExitStack() as ctx:
        const = ctx.enter_context(tc.tile_pool(name="const", bufs=1))
        pers = ctx.enter_context(tc.tile_pool(name="pers", bufs=1))
        psmm = ctx.enter_context(tc.tile_pool(name="psmm", bufs=3, space="PSUM"))
        ptrp = ctx.enter_context(tc.tile_pool(name="ptrp", bufs=1, space="PSUM"))
        pstr = ctx.enter_context(tc.tile_pool(name="pstr", bufs=2, space="PSUM"))

        # ---- constants (idx first: gathers depend on it; then chunk-0 MLP
        # inputs so the pipeline ramps; op weights last)
        idx_sb = const.tile([128, NT * 3], i32)
        nc.sync.dma_start(idx_sb[:], d_idx[:])
        w_sb = {}
        for nm in dw:
            tl = const.tile(list(dw[nm].shape), dw[nm].dtype, tag=nm)
            w_sb[nm] = tl
        q_sb = pers.tile([128, 2, T], mmdt)
        nc.scalar.dma_start(q_sb[:, :, 0:512], d_q[:, :, 0:512])
        early = ["w_so1", "b_so1", "w_aw1", "b_aw1", "w_so2", "brow_so2",
                 "w_aw2", "brow_aw2"]
        for nm in early:
            nc.sync.dma_start(w_sb[nm][:], dw[nm][:])
        cxe_sb = const.tile([128, NT * 64], bf16)
        nc.sync.dma_start(cxe_sb[:], d_cxe[:])
        cye_sb = const.tile([128, NT * 64], bf16)
        nc.sync.dma_start(cye_sb[:], d_cye[:])
        for qd in range(1, NQ):
            nc.sync.dma_start(q_sb[:, :, ts(qd, 512)], d_q[:, :, ts(qd, 512)])
        for nm in dw:
            if nm not in early:
                nc.sync.dma_start(w_sb[nm][:], dw[nm][:])
        ones1 = const.tile([1, 128], mmdt, tag="ones1")
        nc.vector.memset(ones1[:], 1.0)
        negone = const.tile([128, 1], f32, tag="negone")
        nc.vector.memset(negone[:], -1.0)
        ident = const.tile([128, 128], mmdt, tag="ident")
        make_identity(nc, ident[:])
        ones512 = const.tile([1, 512], mmdt, tag="ones512")
        nc.vector.memset(ones512[:], 1.0)

        # ---- persistent activations
        h1so = pers.tile([128, 2, T], mmdt)
        h1aw = pers.tile([128, 2, T], mmdt)
        h1op = pers.tile([128, 2, T], mmdt)
        soT = pers.tile([128, NT * 128], bf16)    # [t, (x64|y64)] pixel offsets
        ew = pers.tile([128, NT * 64], bf16)      # exp(aw logits), (h,p)
        sume = pers.tile([128, NT * 8], f32)
        rec = pers.tile([128, NT * 8], bf16)
        v12 = pers.tile([128, NT * 72], bf16)     # (tile, i, j, h) raw
        v12n = pers.tile([128, NT * 72], bf16)    # normalized
        attn_cm = pers.tile([128, 2, T], mmdt)    # channel-major (c,h)-perm
        out_sb = pers.tile([128, 2, T], f32)

        if repeat > 1:
            ctx.enter_context(tc.For_i(0, repeat, 1))

        # ---- batched gathers (SWDGE on gpsimd); emitted staggered so their
        # transfer timelines don't clog the Pool queue ahead of v12 trees
        patches = ctx.enter_context(tc.tile_pool(name="patch", bufs=1))
        patch_q = []

        def gather(g):
            # 2 token tiles; one single-offset indirect DMA per (tile, window
            # row) -- the only SWDGE addressing pattern verified on hardware
            patch = patches.tile([128, 2 * 3 * 768], pdt, tag=f"patch{g}")
            for c in range(6):
                col = g * 6 + c
                nc.gpsimd.indirect_dma_start(
                    out=patch[:, c * 768:(c + 1) * 768], out_offset=None,
                    in_=d_bev[:],
                    in_offset=bass.IndirectOffsetOnAxis(
                        ap=idx_sb[:, col:col + 1], axis=0))
            patch_q.append(patch)

        gather(0)

        # ---- pipelined stages
        phg = ctx.enter_context(tc.tile_pool(name="phg", bufs=2))
        prodp = ctx.enter_context(tc.tile_pool(name="prod", bufs=3))
        treep = ctx.enter_context(tc.tile_pool(name="tree", bufs=2))
        attnp = ctx.enter_context(tc.tile_pool(name="attn", bufs=3))

        def mlp1(qd, wname, bname, h1):
            """First-layer MLP chunk (512 tokens), channel-major."""
            wt, bt = w_sb[wname], w_sb[bname]
            for m in range(2):
                ps = psmm.tile([128, 512], f32, tag="mmps")
                for kk in range(2):
                    nc.tensor.matmul(
                        ps[:],
                        lhsT=wt[:, kk * 256 + m * 128: kk * 256 + m * 128 + 128],
                        rhs=q_sb[:, kk, ts(qd, 512)],
                        start=(kk == 0), stop=(kk == 1))
                nc.scalar.activation(
                    out=h1[:, m, ts(qd, 512)], in_=ps[:],
                    func=AF.Relu, bias=bt[:, m:m + 1], scale=1.0)

        def so2aw2(j):
            """Token-major so2/aw2 for tile j (lhsT = hidden activations)."""
            ps = pstr.tile([128, 128], f32, tag="so2ps")
            pa = pstr.tile([128, 64], f32, tag="aw2ps")
            nc.tensor.matmul(ps[:], lhsT=ones1[:1, :],
                             rhs=w_sb["brow_so2"][:1, :], start=True, stop=False)
            nc.tensor.matmul(pa[:], lhsT=ones1[:1, :],
                             rhs=w_sb["brow_aw2"][:1, :], start=True, stop=False)
            for kk in range(2):
                last = kk == 1
                nc.tensor.matmul(ps[:], lhsT=h1so[:, kk, ts(j, 128)],
                                 rhs=w_sb["w_so2"][:, kk * 128:(kk + 1) * 128],
                                 start=False, stop=last)
                nc.tensor.matmul(pa[:], lhsT=h1aw[:, kk, ts(j, 128)],
                                 rhs=w_sb["w_aw2"][:, kk * 64:(kk + 1) * 64],
                                 start=False, stop=last)
            nc.scalar.activation(out=soT[:, ts(j, 128)], in_=ps[:],
                                 func=AF.Identity, bias=0.0, scale=1.0)
            nc.scalar.activation(out=ew[:, ts(j, 64)], in_=pa[:],
                                 func=AF.Exp, bias=0.0, scale=1.0)

        def geometry(qd):
            """Per-quarter v12 construction (4 tiles, 256 (tile,h,p) cols)."""
            C = TPQ * 64
            o = qd * C
            dx = phg.tile([128, C], bf16, tag="dx")
            dy = phg.tile([128, C], bf16, tag="dy")
            soTa = soT[:]
            nc.vector.tensor_tensor(
                out=dx[:].rearrange("p (a b) -> p a b", b=64),
                in0=mk_ap(soTa, qd * TPQ * 128, [[128, TPQ], [1, 64]]),
                in1=mk_ap(cxe_sb[:], o, [[64, TPQ], [1, 64]]), op=OP.add)
            nc.vector.tensor_tensor(
                out=dy[:].rearrange("p (a b) -> p a b", b=64),
                in0=mk_ap(soTa, qd * TPQ * 128 + 64, [[128, TPQ], [1, 64]]),
                in1=mk_ap(cye_sb[:], o, [[64, TPQ], [1, 64]]), op=OP.add)

            # x-hats straight into nwx slots [t, (tile, j, h, p)]
            nwx = phg.tile([128, TPQ * 192], bf16, tag="nwx")
            nwxa = nwx[:]
            dx2 = dx[:].rearrange("p (a b) -> p a b", b=64)
            nc.scalar.activation(
                out=mk_ap(nwxa, 0, [[192, TPQ], [1, 64]]), in_=dx2,
                func=AF.Relu, bias=1.0, scale=-1.0)          # h0 = relu(1-dx)
            nc.scalar.activation(
                out=mk_ap(nwxa, 128, [[192, TPQ], [1, 64]]), in_=dx2,
                func=AF.Relu, bias=negone[:], scale=1.0)     # h2 = relu(dx-1)
            u = phg.tile([128, C], bf16, tag="absu")
            nc.scalar.activation(out=u[:], in_=dx[:], func=AF.Abs,
                                 bias=negone[:], scale=1.0)
            nc.scalar.activation(
                out=mk_ap(nwxa, 64, [[192, TPQ], [1, 64]]),
                in_=u[:].rearrange("p (a b) -> p a b", b=64),
                func=AF.Identity, bias=1.0, scale=-1.0)      # h1 = 1-|dx-1|

            # y-hats folded with exp(aw)
            hy0 = phg.tile([128, C], bf16, tag="hy0")
            hy2 = phg.tile([128, C], bf16, tag="hy2")
            nc.scalar.activation(out=hy0[:], in_=dy[:], func=AF.Relu,
                                 bias=1.0, scale=-1.0)
            nc.scalar.activation(out=hy2[:], in_=dy[:], func=AF.Relu,
                                 bias=negone[:], scale=1.0)
            ewq = ew[:, o:o + C]
            ewy0 = phg.tile([128, C], bf16, tag="ewy0")
            ewy1 = phg.tile([128, C], bf16, tag="ewy1")
            ewy2 = phg.tile([128, C], bf16, tag="ewy2")
            nc.vector.tensor_tensor(out=ewy0[:], in0=hy0[:], in1=ewq, op=OP.mult)
            nc.vector.tensor_tensor(out=ewy2[:], in0=hy2[:], in1=ewq, op=OP.mult)
            tmp = phg.tile([128, C], bf16, tag="ewytmp")
            nc.vector.tensor_tensor(out=tmp[:], in0=ewy0[:], in1=ewy2[:],
                                    op=OP.add)
            nc.vector.tensor_tensor(out=ewy1[:], in0=ewq, in1=tmp[:],
                                    op=OP.subtract)

            # softmax denominator and reciprocal
            nc.vector.tensor_reduce(
                out=sume[:, qd * 32:(qd + 1) * 32],
                in_=ewq.rearrange("p (g q) -> p g q", q=NP),
                axis=mybir.AxisListType.X, op=OP.add)
            with nc.allow_low_precision(reason="softmax recip folded to bf16"):
                nc.vector.reciprocal(rec[:, qd * 32:(qd + 1) * 32],
                                     sume[:, qd * 32:(qd + 1) * 32])

            # pr_i = ewy_i (bcast over j) * nwx  -> v12 via gpsimd adds-tree
            v12a = v12[:]
            for i, ewy in enumerate((ewy0, ewy1, ewy2)):
                pr = phg.tile([128, TPQ * 192], bf16, tag=f"pr{i}")
                nc.vector.tensor_tensor(
                    out=pr[:].rearrange("p (a j b) -> p a j b", a=TPQ, j=3),
                    in0=mk_ap(ewy[:], 0, [[64, TPQ], [0, 3], [1, 64]]),
                    in1=mk_ap(nwxa, 0, [[192, TPQ], [64, 3], [1, 64]]),
                    op=OP.mult)
                pra = pr[:]
                t1 = phg.tile([128, TPQ * 96], f32, tag=f"prt1_{i}")
                nc.gpsimd.tensor_tensor(
                    out=t1[:].rearrange("p (g q) -> p g q", q=4),
                    in0=mk_ap(pra, 0, [[8, TPQ * 24], [1, 4]]),
                    in1=mk_ap(pra, 4, [[8, TPQ * 24], [1, 4]]), op=OP.add)
                t1a = t1[:]
                t2 = phg.tile([128, TPQ * 48], f32, tag=f"prt2_{i}")
                nc.gpsimd.tensor_tensor(
                    out=t2[:].rearrange("p (g q) -> p g q", q=2),
                    in0=mk_ap(t1a, 0, [[4, TPQ * 24], [1, 2]]),
                    in1=mk_ap(t1a, 2, [[4, TPQ * 24], [1, 2]]), op=OP.add)
                t2a = t2[:]
                nc.gpsimd.tensor_tensor(
                    out=mk_ap(v12a, qd * TPQ * 72 + i * 24,
                              [[72, TPQ], [1, 24]]),
                    in0=mk_ap(t2a, 0, [[2, TPQ * 24], [1, 1]]),
                    in1=mk_ap(t2a, 1, [[2, TPQ * 24], [1, 1]]), op=OP.add)

            # normalize: v12n = v12 * rec  (bf16 2x)
            nc.vector.tensor_tensor(
                out=v12n[:, qd * TPQ * 72:(qd + 1) * TPQ * 72].rearrange(
                    "p (a s b) -> p a s b", a=TPQ, s=9),
                in0=v12[:, qd * TPQ * 72:(qd + 1) * TPQ * 72].rearrange(
                    "p (a s b) -> p a s b", a=TPQ, s=9),
                in1=mk_ap(rec[:], qd * 32, [[8, TPQ], [0, 9], [1, 8]]),
                op=OP.mult)

        def blend(j):
            """12-pixel weighted blend for tile j -> attn_cm via DMA transpose."""
            jq = j % 2
            patch = patch_q[j // 2]
            peng = nc.gpsimd if j in POOL_PROD_TILES else nc.vector
            teng = nc.gpsimd if j in POOL_TREE_TILES else nc.vector
            prodb = prodp.tile([128, 2304], pdt, tag="prodb")
            peng.tensor_tensor(
                out=prodb[:].rearrange("p (s c b) -> p s c b", s=9, c=32),
                in0=patch[:, jq * 2304:(jq + 1) * 2304].rearrange(
                    "p (s c b) -> p s c b", s=9, c=32),
                in1=mk_ap(v12n[:], j * 72, [[8, 9], [0, 32], [1, 8]]),
                op=OP.mult)
            t1 = treep.tile([128, 1024], pdt, tag="t1")
            teng.tensor_tensor(out=t1[:], in0=prodb[:, 0:1024],
                               in1=prodb[:, 1024:2048], op=OP.add)
            t2 = treep.tile([128, 512], pdt, tag="t2")
            teng.tensor_tensor(out=t2[:], in0=t1[:, 0:512],
                               in1=t1[:, 512:1024], op=OP.add)
            t3 = treep.tile([128, 256], pdt, tag="t3")
            teng.tensor_tensor(out=t3[:], in0=t2[:, 0:256],
                               in1=t2[:, 256:512], op=OP.add)
            attn = attnp.tile([128, 256], pdt, tag="attn")
            teng.tensor_tensor(out=attn[:], in0=t3[:],
                               in1=prodb[:, 2048:2304], op=OP.add)
            if j >= 3 * TPQ:
                # tail tiles: PE transpose + ACT copy (short latency, and
                # keeps the PE p-state warm going into the final op MLP)
                for m in range(2):
                    pt = ptrp.tile([128, 128], pdt, tag="trps")
                    nc.tensor.transpose(pt[:], attn[:, ts(m, 128)], ident[:])
                    nc.vector.tensor_copy(attn_cm[:, m, ts(j, 128)], pt[:])
            else:
                for m in range(2):
                    nc.sync.dma_start_transpose(
                        out=attn_cm[:, m, ts(j, 128)], in_=attn[:, ts(m, 128)])

        def opmlp(t0, nt, dve_acts=False):
            """Output MLP for tokens [t0*128, (t0+nt)*128), channel-major.

            dve_acts: run the bias/activation stage on DVE tensor_scalar
            instead of ACT (for the tail, where DVE is idle and ACT is the
            critical chain)."""
            lo, sz = t0 * 128, nt * 128
            sl = slice(lo, lo + sz)
            for m in range(2):
                ps = psmm.tile([128, 512], f32, tag="mmps")
                for kk in range(2):
                    nc.tensor.matmul(
                        ps[:, 0:sz],
                        lhsT=w_sb["w_op1"][:, kk * 256 + m * 128:
                                           kk * 256 + m * 128 + 128],
                        rhs=attn_cm[:, kk, sl],
                        start=(kk == 0), stop=(kk == 1))
                nc.scalar.activation(
                    out=h1op[:, m, sl], in_=ps[:, 0:sz],
                    func=AF.Relu, bias=w_sb["b_op1"][:, m:m + 1], scale=1.0)
            for m in range(2):
                ps = psmm.tile([128, 512], f32, tag="mmps")
                for kk in range(2):
                    nc.tensor.matmul(
                        ps[:, 0:sz],
                        lhsT=w_sb["w_op2"][:, kk * 256 + m * 128:
                                           kk * 256 + m * 128 + 128],
                        rhs=h1op[:, kk, sl],
                        start=(kk == 0), stop=(kk == 1))
                nc.scalar.activation(
                    out=out_sb[:, m, sl], in_=ps[:, 0:sz],
                    func=AF.Identity, bias=w_sb["b_op2"][:, m:m + 1],
                    scale=1.0)
            nc.sync.dma_start(
                d_out[:, :, sl].rearrange("k p t -> p k t"),
                out_sb[:, :, sl])

        def stage_a(qd):
            mlp1(qd, "w_so1", "b_so1", h1so)
            mlp1(qd, "w_aw1", "b_aw1", h1aw)
            for j in range(qd * TPQ, (qd + 1) * TPQ):
                so2aw2(j)

        # PE p-state warmup: two throwaway matmuls so the first MLP chunk
        # doesn't run at the cold clock
        wps = psmm.tile([128, 512], f32, tag="mmps")
        nc.tensor.matmul(wps[:], lhsT=ones1[:1, :], rhs=ones512[:1, :],
                         start=True, stop=False)
        nc.tensor.matmul(wps[:], lhsT=ones1[:1, :], rhs=ones512[:1, :],
                         start=False, stop=True)

        # software-pipelined emission: keep every engine queue one stage
        # ahead so in-order queues never head-of-line block
        stage_a(0)
        geometry(0)
        stage_a(1)
        gather(1)
        gather(2)
        for j in range(0, TPQ):
            blend(j)
        geometry(1)
        stage_a(2)
        gather(3)
        gather(4)
        gather(5)
        for j in range(TPQ, 2 * TPQ):
            blend(j)
        geometry(2)
        opmlp(0, 4)
        stage_a(3)
        gather(6)
        gather(7)
        for j in range(2 * TPQ, 3 * TPQ):
            blend(j)
        geometry(3)
        opmlp(4, 4)
        for j in range(3 * TPQ, 4 * TPQ):
            blend(j)
            if j == 3 * TPQ + 1:
                opmlp(8, 4)
        opmlp(12, 2)
        opmlp(14, 2, dve_acts=True)

    nc.compile()
    return nc


def get_nc():
    if "nc" not in _CACHE:
        _CACHE["nc"] = _build_nc()
    return _CACHE["nc"]


# ------------------------------------------------------------------- launch
def kernel(**inputs):
    from concourse import bass_utils

    nc = get_nc()
    in_maps = _host_prep(inputs)
    res = bass_utils.run_bass_kernel_spmd(
        nc, in_maps, core_ids=list(range(NCORES)))
    out = np.empty((B, N, D), np.float32)
    for c in range(NCORES):
        b, half = divmod(c, 2)
        o = np.asarray(res.results[c]["out"]).reshape(D, T)
        out[b, half * T:(half + 1) * T, :] = o.T
    return out
